# revision 41
# baseline (speedup 1.0000x reference)
"""Trainium2 Bass kernel for nn_CombinedGNN (gnn_message_passing).

Strategy (8 NeuronCores, node/row parallel, zero collectives):
  - masks[1] in the reference is identically zero (elementwise pow of a 0/1
    matrix), so only mask0 = adj/rowdeg matters.
  - All T=12 timesteps' aggregations are mask0 @ data[t] -> batched into ONE
    matmul  adj @ [X | 1]  with X = data rearranged to [N, 96]; the ones
    column yields row degrees, and the 1/deg row scaling is applied after.
  - Each core owns 625 nodes (padded to 640). It gets adj^T's column block
    (so the contraction dim sits on SBUF partitions with contiguous DMA) and
    computes its nodes' full output independently.
  - The sequential t-chain (his_prev/cur_prev recurrences) runs in
    [feature-on-partition, node-on-free] orientation with host-prepacked /
    permuted weight matrices so no on-chip transposes are needed.
  - adj (exactly representable 0/1) and X are cast to bf16 for the big
    matmul; accumulation is fp32 in PSUM. Everything downstream is fp32.
"""

import numpy as np
import ml_dtypes

import concourse.bass as bass
import concourse.mybir as mybir
import concourse.bass_utils as bass_utils
from concourse.tile import TileContext

# problem constants (hardcoded per harness contract)
N, T, DAY, L = 5000, 12, 8, 2
F = DAY - 1
DIM = T * DAY  # 96
NCORES = 8
NPC = N // NCORES        # 625 nodes per core
NP = 640                 # padded nodes per core
NH = NP // 2             # 320, node half processed per psum chunk
KT = 128                 # contraction tile (partitions; K padded to 5120)
NK = 5120                # padded contraction size
NKT = NK // KT           # 40
KG = 20                  # k-tiles per DMA group
NG = NKT // KG           # 2
XW = DIM + 1             # 97: 96 features + ones column

F32 = mybir.dt.float32
BF16 = mybir.dt.bfloat16
BF16_NP = ml_dtypes.bfloat16

_MAXW = 1


def split_multi_waits(nc):
    """Walrus in this container rejects instructions with >~2 sync waits.
    Hoist extra waits onto preceding single-wait NoOps on the same engine."""
    f = nc.m.functions[0]
    for bb in list(f.blocks):
        new, ctr = [], 0
        for inst in bb.instructions:
            si = inst.sync_info
            waits = list(si.on_wait) if (si and si.on_wait) else []
            if len(waits) > _MAXW:
                head, keep = waits[:-_MAXW], waits[-_MAXW:]
                for i in range(0, len(head), _MAXW):
                    nop = mybir.InstNoOp(
                        name=f"{inst.name}-wsplit{ctr}", engine=inst.engine,
                        ins=[], outs=[],
                        sync_info=mybir.SyncInfo(on_wait=head[i:i + _MAXW],
                                                 on_update=[]),
                    )
                    ctr += 1
                    new.append(nop)
                inst.sync_info = mybir.SyncInfo(
                    on_wait=keep,
                    on_update=list(si.on_update) if si.on_update else [])
            new.append(inst)
        bb.instructions = new


def build_nc():
    nc = bass.Bass()
    a_d = nc.dram_tensor("a", [2, KT, NKT, NH], BF16, kind="ExternalInput")
    xe_d = nc.dram_tensor("xe", [KT, NKT, XW], BF16, kind="ExternalInput")
    dt_d = nc.dram_tensor("dt", [DIM, NP], BF16, kind="ExternalInput")
    pt_d = nc.dram_tensor("pt", [8, T, NP], F32, kind="ExternalInput")
    # w_rp: [104, 96] — rows 0:96 raw block-diag, rows 96:104 prev block
    wrp_d = nc.dram_tensor("wrp", [104, DIM], BF16, kind="ExternalInput")
    wagg_d = nc.dram_tensor("wagg", [DIM, DIM], BF16, kind="ExternalInput")
    # wcomb: [8, T, 104] — cols 0:96 wf block(t), cols 96:104 w2s block(t)
    wcomb_d = nc.dram_tensor("wcomb", [8, T * 104], BF16,
                             kind="ExternalInput")
    out_d = nc.dram_tensor("out", [DIM, NP], F32, kind="ExternalOutput")

    with TileContext(nc) as tc:
        with (
            tc.tile_pool(name="const", bufs=1) as cpool,
            tc.tile_pool(name="adma", bufs=3) as apool,
            tc.tile_pool(name="work", bufs=2) as wpool,
            tc.tile_pool(name="pagg", bufs=1, space="PSUM") as pagg,
            tc.tile_pool(name="pbc", bufs=1, space="PSUM") as pbc,
            tc.tile_pool(name="pp1", bufs=3, space="PSUM") as pp1,
            tc.tile_pool(name="pcm", bufs=2, space="PSUM") as pcm,
        ):
            # SP HWDGE ring order: xe chunk0, a(A,0), xe rest, a(A,1),
            # a(B,*) — phase-1A matmuls can start ~4us in; remaining
            # consts ride the ACT HWDGE ring.
            xe_t = cpool.tile([KT, NKT, XW], BF16)
            a_tiles = {}
            def a_dma(h, g):
                a_t = apool.tile([KT, KG, NH], BF16, tag="a",
                                 name=f"a{h}{g}")
                nc.sync.dma_start(
                    out=a_t, in_=a_d[h, :, g * KG:(g + 1) * KG, :])
                a_tiles[(h, g)] = a_t
            nc.sync.dma_start(out=xe_t[:, 0:KG, :], in_=xe_d[:, 0:KG, :])
            a_dma(0, 0)
            nc.sync.dma_start(out=xe_t[:, KG:NKT, :], in_=xe_d[:, KG:NKT, :])
            a_dma(0, 1)
            a_dma(1, 0)
            a_dma(1, 1)
            # dtprev: rows 0:96 = dataT (static), rows 96:104 = prev state
            dtprev_t = cpool.tile([104, NP], BF16)
            nc.scalar.dma_start(out=dtprev_t[0:DIM, :], in_=dt_d[:, :])
            nc.vector.memset(dtprev_t[DIM:104, :], 0.0)
            pt_t = cpool.tile([8, T, NP], F32)
            nc.scalar.dma_start(out=pt_t, in_=pt_d[:, :, :])
            wrp_t = cpool.tile([104, DIM], BF16)
            nc.scalar.dma_start(out=wrp_t, in_=wrp_d[:, :])
            wagg_t = cpool.tile([DIM, DIM], BF16)
            nc.scalar.dma_start(out=wagg_t, in_=wagg_d[:, :])
            wcomb_t = cpool.tile([8, T * 104], BF16)
            nc.scalar.dma_start(out=wcomb_t, in_=wcomb_d[:, :])

            ones_t = cpool.tile([1, DIM], F32)
            nc.vector.memset(ones_t, 1.0)
            h2_t = cpool.tile([8, T, NP], BF16)
            aggs_t = cpool.tile([DIM, NP], BF16)
            outt_t = cpool.tile([DIM, NP], F32)

            # phase 1: aggT[97, NH] = [X|1]^T @ adjT_shard  per node half
            def phase1(h):
                for g in range(NG):
                    a_t = a_tiles[(h, g)]
                    for j in range(KG):
                        k = g * KG + j
                        nc.tensor.matmul(aggp_t[h], xe_t[:, k, :],
                                         a_t[:, j, :],
                                         start=(k == 0), stop=(k == NKT - 1))

            # row-normalization: broadcast max(deg,1) to 96 partitions via
            # a tiny matmul, then divide (avoids the slow DVE reciprocal).
            def transition(h):
                cs = slice(h * NH, (h + 1) * NH)
                aggp = aggp_t[h]
                degm = wpool.tile([1, NH], F32, tag="degm", name=f"degm{h}")
                nc.vector.tensor_scalar_max(degm, aggp[DIM:DIM + 1, :], 1.0)
                rb_p = pbc.tile([DIM, NH], F32, tag="rbp", name=f"rbp{h}")
                nc.tensor.matmul(rb_p, ones_t, degm, start=True, stop=True)
                rb_s = wpool.tile([DIM, NH], F32, tag="rbs", name=f"rbs{h}")
                nc.vector.reciprocal(rb_s, rb_p)
                nc.vector.tensor_mul(aggs_t[:, cs], aggp[0:DIM, :], rb_s)

            # phase 2: sequential t-chain for one node half.
            # Per t: p1 = wagg[t]^T @ aggs + w_rp[t]^T @ [dt; prev] (chain);
            # pcomb += wcomb[t]^T @ h(t)  (rows 0:96 final acc, 96:104 prev
            # acc, read mid-group by the relu).
            def chain_step(h, t, pcomb):
                cs = slice(h * NH, (h + 1) * NH)
                r8 = slice(t * 8, t * 8 + 8)
                p1 = pp1.tile([8, NH], F32, tag="p1", name=f"p1_{h}_{t}")
                nc.tensor.matmul(p1, wagg_t[:, r8], aggs_t[:, cs],
                                 start=True, stop=False)
                nc.tensor.matmul(p1, wrp_t[:, r8], dtprev_t[:, cs],
                                 start=False, stop=True)
                # h(t) = relu(p1) + pos(t)   (fused on DVE)
                nc.vector.scalar_tensor_tensor(
                    h2_t[:, t, cs], p1, 0.0, pt_t[:, t, cs],
                    op0=mybir.AluOpType.max, op1=mybir.AluOpType.add)
                nc.tensor.matmul(pcomb,
                                 wcomb_t[:, t * 104:(t + 1) * 104],
                                 h2_t[:, t, cs],
                                 start=(t == 0), stop=(t == T - 1),
                                 skip_group_check=True)
                # prev = relu(p2 rows)  (ScalarE, off the DVE)
                if t < T - 1:
                    nc.scalar.activation(
                        dtprev_t[DIM:104, cs], pcomb[DIM:104, :],
                        mybir.ActivationFunctionType.Relu)

            def final(h, pcomb):
                cs = slice(h * NH, (h + 1) * NH)
                nc.scalar.activation(outt_t[:, cs], pcomb[0:DIM, :],
                                     mybir.ActivationFunctionType.Relu)
                nc.sync.dma_start(out=out_d[:, cs], in_=outt_t[:, cs])

            # emission order = scheduler priority: chain A is emitted
            # before phase-1B matmuls so the PE stream interleaves B's
            # matmuls into chain-A's dependency gaps.
            aggp_t = [pagg.tile([XW, NH], F32, tag="aggp", name=f"aggp{h}")
                      for h in range(2)]
            pcombs = [pcm.tile([104, NH], F32, tag="pcm", name=f"pcomb{h}")
                      for h in range(2)]
            phase1(0)
            transition(0)
            for t in range(T):
                chain_step(0, t, pcombs[0])
            phase1(1)
            transition(1)
            for t in range(T):
                chain_step(1, t, pcombs[1])
            final(0, pcombs[0])
            final(1, pcombs[1])

    split_multi_waits(nc)
    return nc


def prep_in_maps(adj, data, pos, his_W, cur_W, his_weight, cur_weight,
                 final_weight):
    adj = np.asarray(adj, dtype=np.float32)
    data = np.asarray(data, dtype=np.float32)
    pos = np.asarray(pos, dtype=np.float32)
    his_W = np.asarray(his_W, dtype=np.float32)
    cur_W = np.asarray(cur_W, dtype=np.float32)
    his_weight = np.asarray(his_weight, dtype=np.float32)
    cur_weight = np.asarray(cur_weight, dtype=np.float32)
    final_weight = np.asarray(final_weight, dtype=np.float32)

    # X = data rearranged [N, 96] (col = t*8+d), plus ones column;
    # contraction dim zero-padded to NK=5120 for full-128-partition tiles
    X = np.ascontiguousarray(data.transpose(1, 0, 2).reshape(N, DIM))
    Xe = np.zeros((NK, XW), np.float32)
    Xe[:N, :DIM] = X
    Xe[:N, DIM] = 1.0
    # pre-tiled for DMA: xe[p, k, c] = Xe[k*KT+p, c]
    xe_h = np.ascontiguousarray(
        Xe.reshape(NKT, KT, XW).transpose(1, 0, 2)).astype(BF16_NP)

    adjT = np.ascontiguousarray(adj.T).astype(BF16_NP)

    # weight packing (zero-padded block maps, see build_nc layout)
    wraw = np.zeros((DIM, DIM), np.float32)
    wagg = np.zeros((DIM, DIM), np.float32)
    wprev = np.zeros((8, DIM), np.float32)
    for t in range(T):
        wraw[t * 8:t * 8 + 7, t * 8:t * 8 + 7] = his_W[t][:, 0:7].T
        wraw[t * 8 + 7, t * 8 + 7] = cur_W[t][0, 0]
        wagg[t * 8:t * 8 + 7, t * 8:t * 8 + 7] = his_W[t][:, 7:14].T
        wagg[t * 8 + 7, t * 8 + 7] = cur_W[t][0, 1]
        wprev[0:7, t * 8:t * 8 + 7] = his_W[t][:, 21:28].T
        wprev[7, t * 8 + 7] = cur_W[t][0, 3]
    # w2s[d, 8t'+o] = prev-update weight from h(t') feature d to output o;
    # t-invariant blocks, accumulated incrementally on-chip.
    w2 = np.zeros((8, DIM), np.float32)
    for tp in range(T):
        w2[0:7, tp * 8:tp * 8 + 7] = his_weight[:, 7 * tp:7 * tp + 7].T
        w2[7, tp * 8 + 7] = cur_weight[0, tp]
    # interleaved feature (8t+d) -> reference feature (7t+d | 84+t)
    f_ref = np.array([7 * t + d if d < 7 else 84 + t
                      for t in range(T) for d in range(8)])
    wf96 = final_weight[:, f_ref].T  # [96 (8t+d), 96 (out)]
    # wf3[d, t*96+o] = wf96[8t+d, o]
    wf = np.ascontiguousarray(
        wf96.reshape(T, 8, DIM).transpose(1, 0, 2).reshape(8, T * DIM))
    # merged lhsT blocks:
    # wrp [104, 96]: rows 0:96 = wraw block-diag, rows 96:104 = wprev
    wrp = np.concatenate([wraw, wprev], axis=0)
    # wcomb [8, T*104]: per t, cols 0:96 = wf block(t), cols 96:104 = w2s(t)
    wcomb = np.zeros((8, T, 104), np.float32)
    for t in range(T):
        wcomb[:, t, 0:DIM] = wf[:, t * DIM:(t + 1) * DIM]
        wcomb[:, t, DIM:104] = w2[:, t * 8:(t + 1) * 8]
    wcomb = np.ascontiguousarray(wcomb.reshape(8, T * 104))

    in_maps = []
    for c in range(NCORES):
        c0 = c * NPC
        ac = np.zeros((NK, NP), BF16_NP)
        ac[:N, :NPC] = adjT[:, c0:c0 + NPC]
        # a[h, p, k, n] = ac[k*KT+p, h*NH+n]
        ah = np.ascontiguousarray(
            ac.reshape(NKT, KT, 2, NH).transpose(2, 1, 0, 3))
        dtc = np.zeros((DIM, NP), np.float32)
        dtc[:, :NPC] = data[:, c0:c0 + NPC, :].transpose(0, 2, 1).reshape(
            DIM, NPC)
        ptc = np.zeros((8, T, NP), np.float32)
        ptc[:, :, :NPC] = pos[:, c0:c0 + NPC, :].transpose(2, 0, 1)
        in_maps.append({
            "a": ah, "xe": xe_h, "dt": dtc.astype(BF16_NP), "pt": ptc,
            "wrp": wrp.astype(BF16_NP), "wagg": wagg.astype(BF16_NP),
            "wcomb": wcomb.astype(BF16_NP),
        })
    return in_maps


def assemble(results):
    out = np.empty((N, DIM), np.float32)
    for c in range(NCORES):
        out[c * NPC:(c + 1) * NPC, :] = results[c]["out"][:, :NPC].T
    return out


_NC_CACHE = None


def get_nc():
    global _NC_CACHE
    if _NC_CACHE is None:
        _NC_CACHE = build_nc()
    return _NC_CACHE


def run_spmd(in_maps, **kwargs):
    nc = get_nc()
    return bass_utils.run_bass_kernel_spmd(
        nc, in_maps, list(range(NCORES)), **kwargs)


def kernel(**inputs):
    in_maps = prep_in_maps(**inputs)
    res = run_spmd(in_maps)
    return assemble(res.results)


# revision 42
# speedup vs baseline: 1.1416x; 1.1416x over previous
"""Trainium2 Bass kernel for nn_CombinedGNN (gnn_message_passing).

Strategy (8 NeuronCores, node/row parallel, zero collectives):
  - masks[1] in the reference is identically zero (elementwise pow of a 0/1
    matrix), so only mask0 = adj/rowdeg matters.
  - All T=12 timesteps' aggregations are mask0 @ data[t] -> batched into ONE
    matmul  adj @ [X | 1]  with X = data rearranged to [N, 96]; the ones
    column yields row degrees, and the 1/deg row scaling is applied after.
  - Each core owns 625 nodes (padded to 640). It gets adj^T's column block
    (so the contraction dim sits on SBUF partitions with contiguous DMA) and
    computes its nodes' full output independently.
  - The sequential t-chain (his_prev/cur_prev recurrences) runs in
    [feature-on-partition, node-on-free] orientation with host-prepacked /
    permuted weight matrices so no on-chip transposes are needed.
  - adj (exactly representable 0/1) and X are cast to bf16 for the big
    matmul; accumulation is fp32 in PSUM. Everything downstream is fp32.
"""

import numpy as np
import ml_dtypes

import concourse.bass as bass
import concourse.mybir as mybir
import concourse.bass_utils as bass_utils
from concourse.tile import TileContext

# problem constants (hardcoded per harness contract)
N, T, DAY, L = 5000, 12, 8, 2
F = DAY - 1
DIM = T * DAY  # 96
NCORES = 8
NPC = N // NCORES        # 625 nodes per core
NP = 640                 # padded nodes per core
NH = NP // 2             # 320, node half processed per psum chunk
KT = 128                 # contraction tile (partitions; K padded to 5120)
NK = 5120                # padded contraction size
NKT = NK // KT           # 40
KG = 20                  # k-tiles per DMA group
NG = NKT // KG           # 2
XW = DIM + 1             # 97: 96 features + ones column

F32 = mybir.dt.float32
BF16 = mybir.dt.bfloat16
BF16_NP = ml_dtypes.bfloat16

_MAXW = 1


def split_multi_waits(nc):
    """Walrus in this container rejects instructions with >~2 sync waits.
    Hoist extra waits onto preceding single-wait NoOps on the same engine."""
    f = nc.m.functions[0]
    for bb in list(f.blocks):
        new, ctr = [], 0
        for inst in bb.instructions:
            si = inst.sync_info
            waits = list(si.on_wait) if (si and si.on_wait) else []
            if len(waits) > _MAXW:
                head, keep = waits[:-_MAXW], waits[-_MAXW:]
                for i in range(0, len(head), _MAXW):
                    nop = mybir.InstNoOp(
                        name=f"{inst.name}-wsplit{ctr}", engine=inst.engine,
                        ins=[], outs=[],
                        sync_info=mybir.SyncInfo(on_wait=head[i:i + _MAXW],
                                                 on_update=[]),
                    )
                    ctr += 1
                    new.append(nop)
                inst.sync_info = mybir.SyncInfo(
                    on_wait=keep,
                    on_update=list(si.on_update) if si.on_update else [])
            new.append(inst)
        bb.instructions = new


def build_nc():
    nc = bass.Bass()
    a_d = nc.dram_tensor("a", [2, KT, NKT, NH], BF16, kind="ExternalInput")
    xe_d = nc.dram_tensor("xe", [KT, NKT, XW], BF16, kind="ExternalInput")
    dt_d = nc.dram_tensor("dt", [DIM, NP], BF16, kind="ExternalInput")
    pt_d = nc.dram_tensor("pt", [8, T, NP], F32, kind="ExternalInput")
    # w_rp: [104, 96] — rows 0:96 raw block-diag, rows 96:104 prev block
    wrp_d = nc.dram_tensor("wrp", [104, DIM], BF16, kind="ExternalInput")
    wagg_d = nc.dram_tensor("wagg", [DIM, DIM], BF16, kind="ExternalInput")
    # wcomb: [8, T, 104] — cols 0:96 wf block(t), cols 96:104 w2s block(t)
    wcomb_d = nc.dram_tensor("wcomb", [8, T * 104], BF16,
                             kind="ExternalInput")
    out_d = nc.dram_tensor("out", [DIM, NP], F32, kind="ExternalOutput")

    with TileContext(nc) as tc:
        with (
            tc.tile_pool(name="const", bufs=1) as cpool,
            tc.tile_pool(name="adma", bufs=3) as apool,
            tc.tile_pool(name="work", bufs=2) as wpool,
            tc.tile_pool(name="pagg", bufs=1, space="PSUM") as pagg,
            tc.tile_pool(name="pbc", bufs=1, space="PSUM") as pbc,
            tc.tile_pool(name="pp1", bufs=3, space="PSUM") as pp1,
            tc.tile_pool(name="pcm", bufs=2, space="PSUM") as pcm,
        ):
            # SP HWDGE ring order: xe chunk0, a(A,0), xe rest, a(A,1),
            # a(B,*) — phase-1A matmuls can start ~4us in; remaining
            # consts ride the ACT HWDGE ring.
            xe_t = cpool.tile([KT, NKT, XW], BF16)
            a_tiles = {}
            def a_dma(h, g):
                a_t = apool.tile([KT, KG, NH], BF16, tag="a",
                                 name=f"a{h}{g}")
                nc.sync.dma_start(
                    out=a_t, in_=a_d[h, :, g * KG:(g + 1) * KG, :])
                a_tiles[(h, g)] = a_t
            nc.sync.dma_start(out=xe_t[:, 0:KG, :], in_=xe_d[:, 0:KG, :])
            a_dma(0, 0)
            nc.sync.dma_start(out=xe_t[:, KG:NKT, :], in_=xe_d[:, KG:NKT, :])
            a_dma(0, 1)
            a_dma(1, 0)
            a_dma(1, 1)
            # dtprev: rows 0:96 = dataT (static), rows 96:104 = prev state
            dtprev_t = cpool.tile([104, NP], BF16)
            nc.scalar.dma_start(out=dtprev_t[0:DIM, :], in_=dt_d[:, :])
            nc.vector.memset(dtprev_t[DIM:104, :], 0.0)
            pt_t = cpool.tile([8, T, NP], F32)
            nc.scalar.dma_start(out=pt_t, in_=pt_d[:, :, :])
            wrp_t = cpool.tile([104, DIM], BF16)
            nc.scalar.dma_start(out=wrp_t, in_=wrp_d[:, :])
            wagg_t = cpool.tile([DIM, DIM], BF16)
            nc.scalar.dma_start(out=wagg_t, in_=wagg_d[:, :])
            wcomb_t = cpool.tile([8, T * 104], BF16)
            nc.scalar.dma_start(out=wcomb_t, in_=wcomb_d[:, :])

            ones_t = cpool.tile([1, DIM], F32)
            nc.vector.memset(ones_t, 1.0)
            h2_t = cpool.tile([8, T, NP], BF16)
            aggs_t = cpool.tile([DIM, NP], BF16)
            outt_t = cpool.tile([DIM, NP], F32)

            # phase 1: aggT[97, NH] = [X|1]^T @ adjT_shard  per node half
            def phase1(h):
                for g in range(NG):
                    a_t = a_tiles[(h, g)]
                    for j in range(KG):
                        k = g * KG + j
                        nc.tensor.matmul(aggp_t[h], xe_t[:, k, :],
                                         a_t[:, j, :],
                                         start=(k == 0), stop=(k == NKT - 1))

            # row-normalization: broadcast max(deg,1) to 96 partitions via
            # a tiny matmul, then divide (avoids the slow DVE reciprocal).
            def transition(h):
                cs = slice(h * NH, (h + 1) * NH)
                aggp = aggp_t[h]
                degm = wpool.tile([1, NH], F32, tag="degm", name=f"degm{h}")
                nc.vector.tensor_scalar_max(degm, aggp[DIM:DIM + 1, :], 1.0)
                rb_p = pbc.tile([DIM, NH], F32, tag="rbp", name=f"rbp{h}")
                nc.tensor.matmul(rb_p, ones_t, degm, start=True, stop=True)
                rb_s = wpool.tile([DIM, NH], F32, tag="rbs", name=f"rbs{h}")
                nc.vector.reciprocal(rb_s, rb_p)
                nc.vector.tensor_mul(aggs_t[:, cs], aggp[0:DIM, :], rb_s)

            # phase 2: sequential t-chain for one node half.
            # Per t: p1 = wagg[t]^T @ aggs + w_rp[t]^T @ [dt; prev] (chain);
            # pcomb += wcomb[t]^T @ h(t)  (rows 0:96 final acc, 96:104 prev
            # acc, read mid-group by the relu).
            def chain_step(h, t, pcomb):
                cs = slice(h * NH, (h + 1) * NH)
                r8 = slice(t * 8, t * 8 + 8)
                p1 = pp1.tile([8, NH], F32, tag="p1", name=f"p1_{h}_{t}")
                nc.tensor.matmul(p1, wagg_t[:, r8], aggs_t[:, cs],
                                 start=True, stop=False)
                nc.tensor.matmul(p1, wrp_t[:, r8], dtprev_t[:, cs],
                                 start=False, stop=True)
                # h(t) = relu(p1) + pos(t)   (fused on DVE)
                nc.vector.scalar_tensor_tensor(
                    h2_t[:, t, cs], p1, 0.0, pt_t[:, t, cs],
                    op0=mybir.AluOpType.max, op1=mybir.AluOpType.add)
                nc.tensor.matmul(pcomb,
                                 wcomb_t[:, t * 104:(t + 1) * 104],
                                 h2_t[:, t, cs],
                                 start=(t == 0), stop=(t == T - 1),
                                 skip_group_check=True)
                # prev = relu(p2 rows)  (ScalarE, off the DVE)
                if t < T - 1:
                    nc.scalar.activation(
                        dtprev_t[DIM:104, cs], pcomb[DIM:104, :],
                        mybir.ActivationFunctionType.Relu)

            def final(h, pcomb):
                cs = slice(h * NH, (h + 1) * NH)
                nc.scalar.activation(outt_t[:, cs], pcomb[0:DIM, :],
                                     mybir.ActivationFunctionType.Relu)
                nc.sync.dma_start(out=out_d[:, cs], in_=outt_t[:, cs])

            # emission order = scheduler priority: chain A is emitted
            # before phase-1B matmuls so the PE stream interleaves B's
            # matmuls into chain-A's dependency gaps.
            aggp_t = [pagg.tile([XW, NH], F32, tag="aggp", name=f"aggp{h}")
                      for h in range(2)]
            pcombs = [pcm.tile([104, NH], F32, tag="pcm", name=f"pcomb{h}")
                      for h in range(2)]
            phase1(0)
            transition(0)
            phase1(1)
            transition(1)
            for t in range(T):
                chain_step(0, t, pcombs[0])
                chain_step(1, t, pcombs[1])
            final(0, pcombs[0])
            final(1, pcombs[1])

    split_multi_waits(nc)
    return nc


def prep_in_maps(adj, data, pos, his_W, cur_W, his_weight, cur_weight,
                 final_weight):
    adj = np.asarray(adj, dtype=np.float32)
    data = np.asarray(data, dtype=np.float32)
    pos = np.asarray(pos, dtype=np.float32)
    his_W = np.asarray(his_W, dtype=np.float32)
    cur_W = np.asarray(cur_W, dtype=np.float32)
    his_weight = np.asarray(his_weight, dtype=np.float32)
    cur_weight = np.asarray(cur_weight, dtype=np.float32)
    final_weight = np.asarray(final_weight, dtype=np.float32)

    # X = data rearranged [N, 96] (col = t*8+d), plus ones column;
    # contraction dim zero-padded to NK=5120 for full-128-partition tiles
    X = np.ascontiguousarray(data.transpose(1, 0, 2).reshape(N, DIM))
    Xe = np.zeros((NK, XW), np.float32)
    Xe[:N, :DIM] = X
    Xe[:N, DIM] = 1.0
    # pre-tiled for DMA: xe[p, k, c] = Xe[k*KT+p, c]
    xe_h = np.ascontiguousarray(
        Xe.reshape(NKT, KT, XW).transpose(1, 0, 2)).astype(BF16_NP)

    adjT = np.ascontiguousarray(adj.T).astype(BF16_NP)

    # weight packing (zero-padded block maps, see build_nc layout)
    wraw = np.zeros((DIM, DIM), np.float32)
    wagg = np.zeros((DIM, DIM), np.float32)
    wprev = np.zeros((8, DIM), np.float32)
    for t in range(T):
        wraw[t * 8:t * 8 + 7, t * 8:t * 8 + 7] = his_W[t][:, 0:7].T
        wraw[t * 8 + 7, t * 8 + 7] = cur_W[t][0, 0]
        wagg[t * 8:t * 8 + 7, t * 8:t * 8 + 7] = his_W[t][:, 7:14].T
        wagg[t * 8 + 7, t * 8 + 7] = cur_W[t][0, 1]
        wprev[0:7, t * 8:t * 8 + 7] = his_W[t][:, 21:28].T
        wprev[7, t * 8 + 7] = cur_W[t][0, 3]
    # w2s[d, 8t'+o] = prev-update weight from h(t') feature d to output o;
    # t-invariant blocks, accumulated incrementally on-chip.
    w2 = np.zeros((8, DIM), np.float32)
    for tp in range(T):
        w2[0:7, tp * 8:tp * 8 + 7] = his_weight[:, 7 * tp:7 * tp + 7].T
        w2[7, tp * 8 + 7] = cur_weight[0, tp]
    # interleaved feature (8t+d) -> reference feature (7t+d | 84+t)
    f_ref = np.array([7 * t + d if d < 7 else 84 + t
                      for t in range(T) for d in range(8)])
    wf96 = final_weight[:, f_ref].T  # [96 (8t+d), 96 (out)]
    # wf3[d, t*96+o] = wf96[8t+d, o]
    wf = np.ascontiguousarray(
        wf96.reshape(T, 8, DIM).transpose(1, 0, 2).reshape(8, T * DIM))
    # merged lhsT blocks:
    # wrp [104, 96]: rows 0:96 = wraw block-diag, rows 96:104 = wprev
    wrp = np.concatenate([wraw, wprev], axis=0)
    # wcomb [8, T*104]: per t, cols 0:96 = wf block(t), cols 96:104 = w2s(t)
    wcomb = np.zeros((8, T, 104), np.float32)
    for t in range(T):
        wcomb[:, t, 0:DIM] = wf[:, t * DIM:(t + 1) * DIM]
        wcomb[:, t, DIM:104] = w2[:, t * 8:(t + 1) * 8]
    wcomb = np.ascontiguousarray(wcomb.reshape(8, T * 104))

    in_maps = []
    for c in range(NCORES):
        c0 = c * NPC
        ac = np.zeros((NK, NP), BF16_NP)
        ac[:N, :NPC] = adjT[:, c0:c0 + NPC]
        # a[h, p, k, n] = ac[k*KT+p, h*NH+n]
        ah = np.ascontiguousarray(
            ac.reshape(NKT, KT, 2, NH).transpose(2, 1, 0, 3))
        dtc = np.zeros((DIM, NP), np.float32)
        dtc[:, :NPC] = data[:, c0:c0 + NPC, :].transpose(0, 2, 1).reshape(
            DIM, NPC)
        ptc = np.zeros((8, T, NP), np.float32)
        ptc[:, :, :NPC] = pos[:, c0:c0 + NPC, :].transpose(2, 0, 1)
        in_maps.append({
            "a": ah, "xe": xe_h, "dt": dtc.astype(BF16_NP), "pt": ptc,
            "wrp": wrp.astype(BF16_NP), "wagg": wagg.astype(BF16_NP),
            "wcomb": wcomb.astype(BF16_NP),
        })
    return in_maps


def assemble(results):
    out = np.empty((N, DIM), np.float32)
    for c in range(NCORES):
        out[c * NPC:(c + 1) * NPC, :] = results[c]["out"][:, :NPC].T
    return out


_NC_CACHE = None


def get_nc():
    global _NC_CACHE
    if _NC_CACHE is None:
        _NC_CACHE = build_nc()
    return _NC_CACHE


def run_spmd(in_maps, **kwargs):
    nc = get_nc()
    return bass_utils.run_bass_kernel_spmd(
        nc, in_maps, list(range(NCORES)), **kwargs)


def kernel(**inputs):
    in_maps = prep_in_maps(**inputs)
    res = run_spmd(in_maps)
    return assemble(res.results)


# revision 43
# speedup vs baseline: 1.2949x; 1.1343x over previous
"""Trainium2 Bass kernel for nn_CombinedGNN (gnn_message_passing).

Strategy (8 NeuronCores, node/row parallel, zero collectives):
  - masks[1] in the reference is identically zero (elementwise pow of a 0/1
    matrix), so only mask0 = adj/rowdeg matters.
  - All T=12 timesteps' aggregations are mask0 @ data[t] -> batched into ONE
    matmul  adj @ [X | 1]  with X = data rearranged to [N, 96]; the ones
    column yields row degrees, and the 1/deg row scaling is applied after.
  - Each core owns 625 nodes (padded to 640). It gets adj^T's column block
    (so the contraction dim sits on SBUF partitions with contiguous DMA) and
    computes its nodes' full output independently.
  - The sequential t-chain (his_prev/cur_prev recurrences) runs in
    [feature-on-partition, node-on-free] orientation with host-prepacked /
    permuted weight matrices so no on-chip transposes are needed.
  - adj (exactly representable 0/1) and X are cast to bf16 for the big
    matmul; accumulation is fp32 in PSUM. Everything downstream is fp32.
"""

import numpy as np
import ml_dtypes

import concourse.bass as bass
import concourse.mybir as mybir
import concourse.bass_utils as bass_utils
from concourse.tile import TileContext

# problem constants (hardcoded per harness contract)
N, T, DAY, L = 5000, 12, 8, 2
F = DAY - 1
DIM = T * DAY  # 96
NCORES = 8
NPC = N // NCORES        # 625 nodes per core
NP = 640                 # padded nodes per core
NH = NP // 2             # 320, node half processed per psum chunk
KT = 128                 # contraction tile (partitions; K padded to 5120)
NK = 5120                # padded contraction size
NKT = NK // KT           # 40
KG = 20                  # k-tiles per DMA group
NG = NKT // KG           # 2
XW = DIM + 1             # 97: 96 features + ones column

F32 = mybir.dt.float32
BF16 = mybir.dt.bfloat16
BF16_NP = ml_dtypes.bfloat16

_MAXW = 1


def split_multi_waits(nc):
    """Walrus in this container rejects instructions with >~2 sync waits.
    Hoist extra waits onto preceding single-wait NoOps on the same engine."""
    f = nc.m.functions[0]
    for bb in list(f.blocks):
        new, ctr = [], 0
        for inst in bb.instructions:
            si = inst.sync_info
            waits = list(si.on_wait) if (si and si.on_wait) else []
            if len(waits) > _MAXW:
                head, keep = waits[:-_MAXW], waits[-_MAXW:]
                for i in range(0, len(head), _MAXW):
                    nop = mybir.InstNoOp(
                        name=f"{inst.name}-wsplit{ctr}", engine=inst.engine,
                        ins=[], outs=[],
                        sync_info=mybir.SyncInfo(on_wait=head[i:i + _MAXW],
                                                 on_update=[]),
                    )
                    ctr += 1
                    new.append(nop)
                inst.sync_info = mybir.SyncInfo(
                    on_wait=keep,
                    on_update=list(si.on_update) if si.on_update else [])
            new.append(inst)
        bb.instructions = new


def build_nc():
    nc = bass.Bass()
    a_d = nc.dram_tensor("a", [2, KT, NKT, NH], BF16, kind="ExternalInput")
    xe_d = nc.dram_tensor("xe", [KT, NKT, XW], BF16, kind="ExternalInput")
    dt_d = nc.dram_tensor("dt", [DIM, NP], BF16, kind="ExternalInput")
    pt_d = nc.dram_tensor("pt", [8, T, NP], F32, kind="ExternalInput")
    # w_rp: [104, 96] — rows 0:96 raw block-diag, rows 96:104 prev block
    wrp_d = nc.dram_tensor("wrp", [104, DIM], BF16, kind="ExternalInput")
    wagg_d = nc.dram_tensor("wagg", [DIM, DIM], BF16, kind="ExternalInput")
    # wcomb: [8, T, 104] — cols 0:96 wf block(t), cols 96:104 w2s block(t)
    wcomb_d = nc.dram_tensor("wcomb", [8, T * 104], BF16,
                             kind="ExternalInput")
    out_d = nc.dram_tensor("out", [DIM, NP], F32, kind="ExternalOutput")

    with TileContext(nc) as tc:
        with (
            tc.tile_pool(name="const", bufs=1) as cpool,
            tc.tile_pool(name="adma", bufs=3) as apool,
            tc.tile_pool(name="work", bufs=2) as wpool,
            tc.tile_pool(name="pagg", bufs=1, space="PSUM") as pagg,
            tc.tile_pool(name="pbc", bufs=1, space="PSUM") as pbc,
            tc.tile_pool(name="pp1", bufs=3, space="PSUM") as pp1,
            tc.tile_pool(name="pcm", bufs=2, space="PSUM") as pcm,
        ):
            # SP HWDGE ring order: xe chunk0, a(A,0), xe rest, a(A,1),
            # a(B,*) — phase-1A matmuls can start ~4us in; remaining
            # consts ride the ACT HWDGE ring.
            xe_t = cpool.tile([KT, NKT, XW], BF16)
            a_tiles = {}
            def a_dma(h, g):
                a_t = apool.tile([KT, KG, NH], BF16, tag="a",
                                 name=f"a{h}{g}")
                nc.sync.dma_start(
                    out=a_t, in_=a_d[h, :, g * KG:(g + 1) * KG, :])
                a_tiles[(h, g)] = a_t
            nc.sync.dma_start(out=xe_t[:, 0:KG, :], in_=xe_d[:, 0:KG, :])
            nc.sync.dma_start(out=xe_t[:, KG:NKT, :], in_=xe_d[:, KG:NKT, :])
            a_dma(0, 0)
            a_dma(0, 1)
            a_dma(1, 0)
            a_dma(1, 1)
            # dtprev: rows 0:96 = dataT (static), rows 96:104 = prev state
            dtprev_t = cpool.tile([104, NP], BF16)
            nc.scalar.dma_start(out=dtprev_t[0:DIM, :], in_=dt_d[:, :])
            nc.vector.memset(dtprev_t[DIM:104, :], 0.0)
            pt_t = cpool.tile([8, T, NP], F32)
            nc.scalar.dma_start(out=pt_t, in_=pt_d[:, :, :])
            wrp_t = cpool.tile([104, DIM], BF16)
            nc.scalar.dma_start(out=wrp_t, in_=wrp_d[:, :])
            wagg_t = cpool.tile([DIM, DIM], BF16)
            nc.scalar.dma_start(out=wagg_t, in_=wagg_d[:, :])
            wcomb_t = cpool.tile([8, T * 104], BF16)
            nc.scalar.dma_start(out=wcomb_t, in_=wcomb_d[:, :])

            ones_t = cpool.tile([1, DIM], F32)
            nc.vector.memset(ones_t, 1.0)
            h2_t = cpool.tile([8, T, NP], BF16)
            aggs_t = cpool.tile([DIM, NP], BF16)
            outt_t = cpool.tile([DIM, NP], F32)

            # phase 1: aggT[97, NH] = [X|1]^T @ adjT_shard  per node half
            def phase1(h):
                for g in range(NG):
                    a_t = a_tiles[(h, g)]
                    for j in range(KG):
                        k = g * KG + j
                        nc.tensor.matmul(aggp_t[h], xe_t[:, k, :],
                                         a_t[:, j, :],
                                         start=(k == 0), stop=(k == NKT - 1))

            # row-normalization: broadcast max(deg,1) to 96 partitions via
            # a tiny matmul, then divide (avoids the slow DVE reciprocal).
            def transition(h):
                cs = slice(h * NH, (h + 1) * NH)
                aggp = aggp_t[h]
                degm = wpool.tile([1, NH], F32, tag="degm", name=f"degm{h}")
                nc.vector.tensor_scalar_max(degm, aggp[DIM:DIM + 1, :], 1.0)
                rb_p = pbc.tile([DIM, NH], F32, tag="rbp", name=f"rbp{h}")
                nc.tensor.matmul(rb_p, ones_t, degm, start=True, stop=True)
                rb_s = wpool.tile([DIM, NH], F32, tag="rbs", name=f"rbs{h}")
                nc.vector.reciprocal(rb_s, rb_p)
                nc.vector.tensor_mul(aggs_t[:, cs], aggp[0:DIM, :], rb_s)

            # phase 2: sequential t-chain for one node half.
            # Per t: p1 = wagg[t]^T @ aggs + w_rp[t]^T @ [dt; prev] (chain);
            # pcomb += wcomb[t]^T @ h(t)  (rows 0:96 final acc, 96:104 prev
            # acc, read mid-group by the relu).
            def chain_step(h, t, pcomb):
                cs = slice(h * NH, (h + 1) * NH)
                r8 = slice(t * 8, t * 8 + 8)
                p1 = pp1.tile([8, NH], F32, tag="p1", name=f"p1_{h}_{t}")
                nc.tensor.matmul(p1, wagg_t[:, r8], aggs_t[:, cs],
                                 start=True, stop=False)
                nc.tensor.matmul(p1, wrp_t[:, r8], dtprev_t[:, cs],
                                 start=False, stop=True)
                # h(t) = relu(p1) + pos(t)   (fused on DVE)
                nc.vector.scalar_tensor_tensor(
                    h2_t[:, t, cs], p1, 0.0, pt_t[:, t, cs],
                    op0=mybir.AluOpType.max, op1=mybir.AluOpType.add)
                nc.tensor.matmul(pcomb,
                                 wcomb_t[:, t * 104:(t + 1) * 104],
                                 h2_t[:, t, cs],
                                 start=(t == 0), stop=(t == T - 1),
                                 skip_group_check=True)
                # prev = relu(p2 rows)  (ScalarE, off the DVE)
                if t < T - 1:
                    nc.scalar.activation(
                        dtprev_t[DIM:104, cs], pcomb[DIM:104, :],
                        mybir.ActivationFunctionType.Relu)

            def final(h, pcomb):
                cs = slice(h * NH, (h + 1) * NH)
                nc.scalar.activation(outt_t[:, cs], pcomb[0:DIM, :],
                                     mybir.ActivationFunctionType.Relu)
                nc.sync.dma_start(out=out_d[:, cs], in_=outt_t[:, cs])

            # emission order = scheduler priority: chain A is emitted
            # before phase-1B matmuls so the PE stream interleaves B's
            # matmuls into chain-A's dependency gaps.
            aggp_t = [pagg.tile([XW, NH], F32, tag="aggp", name=f"aggp{h}")
                      for h in range(2)]
            pcombs = [pcm.tile([104, NH], F32, tag="pcm", name=f"pcomb{h}")
                      for h in range(2)]
            phase1(0)
            transition(0)
            phase1(1)
            transition(1)
            for t in range(T):
                chain_step(0, t, pcombs[0])
                chain_step(1, t, pcombs[1])
            final(0, pcombs[0])
            final(1, pcombs[1])

    split_multi_waits(nc)
    return nc


def prep_in_maps(adj, data, pos, his_W, cur_W, his_weight, cur_weight,
                 final_weight):
    adj = np.asarray(adj, dtype=np.float32)
    data = np.asarray(data, dtype=np.float32)
    pos = np.asarray(pos, dtype=np.float32)
    his_W = np.asarray(his_W, dtype=np.float32)
    cur_W = np.asarray(cur_W, dtype=np.float32)
    his_weight = np.asarray(his_weight, dtype=np.float32)
    cur_weight = np.asarray(cur_weight, dtype=np.float32)
    final_weight = np.asarray(final_weight, dtype=np.float32)

    # X = data rearranged [N, 96] (col = t*8+d), plus ones column;
    # contraction dim zero-padded to NK=5120 for full-128-partition tiles
    X = np.ascontiguousarray(data.transpose(1, 0, 2).reshape(N, DIM))
    Xe = np.zeros((NK, XW), np.float32)
    Xe[:N, :DIM] = X
    Xe[:N, DIM] = 1.0
    # pre-tiled for DMA: xe[p, k, c] = Xe[k*KT+p, c]
    xe_h = np.ascontiguousarray(
        Xe.reshape(NKT, KT, XW).transpose(1, 0, 2)).astype(BF16_NP)

    adjT = np.ascontiguousarray(adj.T).astype(BF16_NP)

    # weight packing (zero-padded block maps, see build_nc layout)
    wraw = np.zeros((DIM, DIM), np.float32)
    wagg = np.zeros((DIM, DIM), np.float32)
    wprev = np.zeros((8, DIM), np.float32)
    for t in range(T):
        wraw[t * 8:t * 8 + 7, t * 8:t * 8 + 7] = his_W[t][:, 0:7].T
        wraw[t * 8 + 7, t * 8 + 7] = cur_W[t][0, 0]
        wagg[t * 8:t * 8 + 7, t * 8:t * 8 + 7] = his_W[t][:, 7:14].T
        wagg[t * 8 + 7, t * 8 + 7] = cur_W[t][0, 1]
        wprev[0:7, t * 8:t * 8 + 7] = his_W[t][:, 21:28].T
        wprev[7, t * 8 + 7] = cur_W[t][0, 3]
    # w2s[d, 8t'+o] = prev-update weight from h(t') feature d to output o;
    # t-invariant blocks, accumulated incrementally on-chip.
    w2 = np.zeros((8, DIM), np.float32)
    for tp in range(T):
        w2[0:7, tp * 8:tp * 8 + 7] = his_weight[:, 7 * tp:7 * tp + 7].T
        w2[7, tp * 8 + 7] = cur_weight[0, tp]
    # interleaved feature (8t+d) -> reference feature (7t+d | 84+t)
    f_ref = np.array([7 * t + d if d < 7 else 84 + t
                      for t in range(T) for d in range(8)])
    wf96 = final_weight[:, f_ref].T  # [96 (8t+d), 96 (out)]
    # wf3[d, t*96+o] = wf96[8t+d, o]
    wf = np.ascontiguousarray(
        wf96.reshape(T, 8, DIM).transpose(1, 0, 2).reshape(8, T * DIM))
    # merged lhsT blocks:
    # wrp [104, 96]: rows 0:96 = wraw block-diag, rows 96:104 = wprev
    wrp = np.concatenate([wraw, wprev], axis=0)
    # wcomb [8, T*104]: per t, cols 0:96 = wf block(t), cols 96:104 = w2s(t)
    wcomb = np.zeros((8, T, 104), np.float32)
    for t in range(T):
        wcomb[:, t, 0:DIM] = wf[:, t * DIM:(t + 1) * DIM]
        wcomb[:, t, DIM:104] = w2[:, t * 8:(t + 1) * 8]
    wcomb = np.ascontiguousarray(wcomb.reshape(8, T * 104))

    in_maps = []
    for c in range(NCORES):
        c0 = c * NPC
        ac = np.zeros((NK, NP), BF16_NP)
        ac[:N, :NPC] = adjT[:, c0:c0 + NPC]
        # a[h, p, k, n] = ac[k*KT+p, h*NH+n]
        ah = np.ascontiguousarray(
            ac.reshape(NKT, KT, 2, NH).transpose(2, 1, 0, 3))
        dtc = np.zeros((DIM, NP), np.float32)
        dtc[:, :NPC] = data[:, c0:c0 + NPC, :].transpose(0, 2, 1).reshape(
            DIM, NPC)
        ptc = np.zeros((8, T, NP), np.float32)
        ptc[:, :, :NPC] = pos[:, c0:c0 + NPC, :].transpose(2, 0, 1)
        in_maps.append({
            "a": ah, "xe": xe_h, "dt": dtc.astype(BF16_NP), "pt": ptc,
            "wrp": wrp.astype(BF16_NP), "wagg": wagg.astype(BF16_NP),
            "wcomb": wcomb.astype(BF16_NP),
        })
    return in_maps


def assemble(results):
    out = np.empty((N, DIM), np.float32)
    for c in range(NCORES):
        out[c * NPC:(c + 1) * NPC, :] = results[c]["out"][:, :NPC].T
    return out


_NC_CACHE = None


def get_nc():
    global _NC_CACHE
    if _NC_CACHE is None:
        _NC_CACHE = build_nc()
    return _NC_CACHE


def run_spmd(in_maps, **kwargs):
    nc = get_nc()
    return bass_utils.run_bass_kernel_spmd(
        nc, in_maps, list(range(NCORES)), **kwargs)


def kernel(**inputs):
    in_maps = prep_in_maps(**inputs)
    res = run_spmd(in_maps)
    return assemble(res.results)


# revision 44
# speedup vs baseline: 1.3882x; 1.0720x over previous
"""Trainium2 Bass kernel for nn_CombinedGNN (gnn_message_passing).

Strategy (8 NeuronCores, node/row parallel, zero collectives):
  - masks[1] in the reference is identically zero (elementwise pow of a 0/1
    matrix), so only mask0 = adj/rowdeg matters.
  - All T=12 timesteps' aggregations are mask0 @ data[t] -> batched into ONE
    matmul  adj @ [X | 1]  with X = data rearranged to [N, 96]; the ones
    column yields row degrees, and the 1/deg row scaling is applied after.
  - Each core owns 625 nodes (padded to 640). It gets adj^T's column block
    (so the contraction dim sits on SBUF partitions with contiguous DMA) and
    computes its nodes' full output independently.
  - The sequential t-chain (his_prev/cur_prev recurrences) runs in
    [feature-on-partition, node-on-free] orientation with host-prepacked /
    permuted weight matrices so no on-chip transposes are needed.
  - adj (exactly representable 0/1) and X are cast to bf16 for the big
    matmul; accumulation is fp32 in PSUM. Everything downstream is fp32.
"""

import numpy as np
import ml_dtypes

import concourse.bass as bass
import concourse.mybir as mybir
import concourse.bass_utils as bass_utils
from concourse.tile import TileContext

# problem constants (hardcoded per harness contract)
N, T, DAY, L = 5000, 12, 8, 2
F = DAY - 1
DIM = T * DAY  # 96
NCORES = 8
NPC = N // NCORES        # 625 nodes per core
NP = 640                 # padded nodes per core
NH = NP // 2             # 320, node half processed per psum chunk
KT = 128                 # contraction tile (partitions; K padded to 5120)
NK = 5120                # padded contraction size
NKT = NK // KT           # 40
KG = 20                  # k-tiles per DMA group
NG = NKT // KG           # 2
XW = DIM + 1             # 97: 96 features + ones column

F32 = mybir.dt.float32
BF16 = mybir.dt.bfloat16
BF16_NP = ml_dtypes.bfloat16

_MAXW = 1


def split_multi_waits(nc):
    """Walrus in this container rejects instructions with >~2 sync waits.
    Hoist extra waits onto preceding single-wait NoOps on the same engine."""
    f = nc.m.functions[0]
    for bb in list(f.blocks):
        new, ctr = [], 0
        for inst in bb.instructions:
            si = inst.sync_info
            waits = list(si.on_wait) if (si and si.on_wait) else []
            if len(waits) > _MAXW:
                head, keep = waits[:-_MAXW], waits[-_MAXW:]
                for i in range(0, len(head), _MAXW):
                    nop = mybir.InstNoOp(
                        name=f"{inst.name}-wsplit{ctr}", engine=inst.engine,
                        ins=[], outs=[],
                        sync_info=mybir.SyncInfo(on_wait=head[i:i + _MAXW],
                                                 on_update=[]),
                    )
                    ctr += 1
                    new.append(nop)
                inst.sync_info = mybir.SyncInfo(
                    on_wait=keep,
                    on_update=list(si.on_update) if si.on_update else [])
            new.append(inst)
        bb.instructions = new


def build_nc():
    nc = bass.Bass()
    a_d = nc.dram_tensor("a", [2, KT, NKT, NH], BF16, kind="ExternalInput")
    xe_d = nc.dram_tensor("xe", [KT, NKT, XW], BF16, kind="ExternalInput")
    dt_d = nc.dram_tensor("dt", [DIM, NP], BF16, kind="ExternalInput")
    pt_d = nc.dram_tensor("pt", [8, T, NP], F32, kind="ExternalInput")
    # w_rp: [104, 96] — rows 0:96 raw block-diag, rows 96:104 prev block
    wrp_d = nc.dram_tensor("wrp", [104, DIM], BF16, kind="ExternalInput")
    wagg_d = nc.dram_tensor("wagg", [DIM, DIM], BF16, kind="ExternalInput")
    # wcomb: [8, T, 104] — cols 0:96 wf block(t), cols 96:104 w2s block(t)
    wcomb_d = nc.dram_tensor("wcomb", [8, T * 104], BF16,
                             kind="ExternalInput")
    out_d = nc.dram_tensor("out", [DIM, NP], F32, kind="ExternalOutput")

    with TileContext(nc) as tc:
        with (
            tc.tile_pool(name="const", bufs=1) as cpool,
            tc.tile_pool(name="adma", bufs=3) as apool,
            tc.tile_pool(name="work", bufs=2) as wpool,
            tc.tile_pool(name="pagg", bufs=2, space="PSUM") as pagg,
            tc.tile_pool(name="pbc", bufs=1, space="PSUM") as pbc,
            tc.tile_pool(name="pp1", bufs=3, space="PSUM") as pp1,
            tc.tile_pool(name="pcm", bufs=2, space="PSUM") as pcm,
        ):
            # SP HWDGE ring order: xe chunk0, a(A,0), xe rest, a(A,1),
            # a(B,*) — phase-1A matmuls can start ~4us in; remaining
            # consts ride the ACT HWDGE ring.
            xe_t = cpool.tile([KT, NKT, XW], BF16)
            a_tiles = {}
            def a_dma(h, g):
                a_t = apool.tile([KT, KG, NH], BF16, tag="a",
                                 name=f"a{h}{g}")
                nc.sync.dma_start(
                    out=a_t, in_=a_d[h, :, g * KG:(g + 1) * KG, :])
                a_tiles[(h, g)] = a_t
            nc.sync.dma_start(out=xe_t[:, 0:KG, :], in_=xe_d[:, 0:KG, :])
            nc.sync.dma_start(out=xe_t[:, KG:NKT, :], in_=xe_d[:, KG:NKT, :])
            a_dma(0, 0)
            a_dma(0, 1)
            a_dma(1, 0)
            a_dma(1, 1)
            # dtprev: rows 0:96 = dataT (static), rows 96:104 = prev state
            dtprev_t = cpool.tile([104, NP], BF16)
            nc.scalar.dma_start(out=dtprev_t[0:DIM, :], in_=dt_d[:, :])
            nc.vector.memset(dtprev_t[DIM:104, :], 0.0)
            pt_t = cpool.tile([8, T, NP], F32)
            nc.scalar.dma_start(out=pt_t, in_=pt_d[:, :, :])
            wrp_t = cpool.tile([104, DIM], BF16)
            nc.scalar.dma_start(out=wrp_t, in_=wrp_d[:, :])
            wagg_t = cpool.tile([DIM, DIM], BF16)
            nc.scalar.dma_start(out=wagg_t, in_=wagg_d[:, :])
            wcomb_t = cpool.tile([8, T * 104], BF16)
            nc.scalar.dma_start(out=wcomb_t, in_=wcomb_d[:, :])

            ones_t = cpool.tile([1, DIM], F32)
            nc.vector.memset(ones_t, 1.0)
            h2_t = cpool.tile([8, T, NP], BF16)
            aggs_t = cpool.tile([DIM, NP], BF16)
            outt_t = cpool.tile([DIM, NP], F32)

            # phase 1: aggT[97, NH] = [X|1]^T @ adjT_shard  per node half
            def phase1(h):
                for g in range(NG):
                    a_t = a_tiles[(h, g)]
                    for j in range(KG):
                        k = g * KG + j
                        nc.tensor.matmul(aggp_t[h], xe_t[:, k, :],
                                         a_t[:, j, :],
                                         start=(k == 0), stop=(k == NKT - 1))

            # row-normalization: broadcast max(deg,1) to 96 partitions via
            # a tiny matmul, then divide (avoids the slow DVE reciprocal).
            def transition(h):
                cs = slice(h * NH, (h + 1) * NH)
                aggp = aggp_t[h]
                degm = wpool.tile([1, NH], F32, tag="degm", name=f"degm{h}")
                nc.vector.tensor_scalar_max(degm, aggp[DIM:DIM + 1, :], 1.0)
                rb_p = pbc.tile([DIM, NH], F32, tag="rbp", name=f"rbp{h}")
                nc.tensor.matmul(rb_p, ones_t, degm, start=True, stop=True)
                rb_s = wpool.tile([DIM, NH], F32, tag="rbs", name=f"rbs{h}")
                nc.vector.reciprocal(rb_s, rb_p)
                nc.vector.tensor_mul(aggs_t[:, cs], aggp[0:DIM, :], rb_s)

            # phase 2: sequential t-chain for one node half.
            # Per t: p1 = wagg[t]^T @ aggs + w_rp[t]^T @ [dt; prev] (chain);
            # pcomb += wcomb[t]^T @ h(t)  (rows 0:96 final acc, 96:104 prev
            # acc, read mid-group by the relu).
            def chain_step(h, t, pcomb):
                cs = slice(h * NH, (h + 1) * NH)
                r8 = slice(t * 8, t * 8 + 8)
                p1 = pp1.tile([8, NH], F32, tag="p1", name=f"p1_{h}_{t}")
                nc.tensor.matmul(p1, wagg_t[:, r8], aggs_t[:, cs],
                                 start=True, stop=False)
                nc.tensor.matmul(p1, wrp_t[:, r8], dtprev_t[:, cs],
                                 start=False, stop=True)
                # h(t) = relu(p1) + pos(t)   (fused on DVE)
                nc.vector.scalar_tensor_tensor(
                    h2_t[:, t, cs], p1, 0.0, pt_t[:, t, cs],
                    op0=mybir.AluOpType.max, op1=mybir.AluOpType.add)
                nc.tensor.matmul(pcomb,
                                 wcomb_t[:, t * 104:(t + 1) * 104],
                                 h2_t[:, t, cs],
                                 start=(t == 0), stop=(t == T - 1),
                                 skip_group_check=True)
                # prev = relu(p2 rows)  (ScalarE, off the DVE)
                if t < T - 1:
                    nc.scalar.activation(
                        dtprev_t[DIM:104, cs], pcomb[DIM:104, :],
                        mybir.ActivationFunctionType.Relu)

            def final(h, pcomb):
                cs = slice(h * NH, (h + 1) * NH)
                nc.scalar.activation(outt_t[:, cs], pcomb[0:DIM, :],
                                     mybir.ActivationFunctionType.Relu)
                nc.sync.dma_start(out=out_d[:, cs], in_=outt_t[:, cs])

            # emission order = scheduler priority: chain A is emitted
            # before phase-1B matmuls so the PE stream interleaves B's
            # matmuls into chain-A's dependency gaps.
            aggp_t = [pagg.tile([XW, NH], F32, tag="aggp", name=f"aggp{h}")
                      for h in range(2)]
            pcombs = [pcm.tile([104, NH], F32, tag="pcm", name=f"pcomb{h}")
                      for h in range(2)]
            phase1(0)
            transition(0)
            phase1(1)
            transition(1)
            for t in range(T):
                chain_step(0, t, pcombs[0])
                chain_step(1, t, pcombs[1])
            final(0, pcombs[0])
            final(1, pcombs[1])

    split_multi_waits(nc)
    return nc


def prep_in_maps(adj, data, pos, his_W, cur_W, his_weight, cur_weight,
                 final_weight):
    adj = np.asarray(adj, dtype=np.float32)
    data = np.asarray(data, dtype=np.float32)
    pos = np.asarray(pos, dtype=np.float32)
    his_W = np.asarray(his_W, dtype=np.float32)
    cur_W = np.asarray(cur_W, dtype=np.float32)
    his_weight = np.asarray(his_weight, dtype=np.float32)
    cur_weight = np.asarray(cur_weight, dtype=np.float32)
    final_weight = np.asarray(final_weight, dtype=np.float32)

    # X = data rearranged [N, 96] (col = t*8+d), plus ones column;
    # contraction dim zero-padded to NK=5120 for full-128-partition tiles
    X = np.ascontiguousarray(data.transpose(1, 0, 2).reshape(N, DIM))
    Xe = np.zeros((NK, XW), np.float32)
    Xe[:N, :DIM] = X
    Xe[:N, DIM] = 1.0
    # pre-tiled for DMA: xe[p, k, c] = Xe[k*KT+p, c]
    xe_h = np.ascontiguousarray(
        Xe.reshape(NKT, KT, XW).transpose(1, 0, 2)).astype(BF16_NP)

    adjT = np.ascontiguousarray(adj.T).astype(BF16_NP)

    # weight packing (zero-padded block maps, see build_nc layout)
    wraw = np.zeros((DIM, DIM), np.float32)
    wagg = np.zeros((DIM, DIM), np.float32)
    wprev = np.zeros((8, DIM), np.float32)
    for t in range(T):
        wraw[t * 8:t * 8 + 7, t * 8:t * 8 + 7] = his_W[t][:, 0:7].T
        wraw[t * 8 + 7, t * 8 + 7] = cur_W[t][0, 0]
        wagg[t * 8:t * 8 + 7, t * 8:t * 8 + 7] = his_W[t][:, 7:14].T
        wagg[t * 8 + 7, t * 8 + 7] = cur_W[t][0, 1]
        wprev[0:7, t * 8:t * 8 + 7] = his_W[t][:, 21:28].T
        wprev[7, t * 8 + 7] = cur_W[t][0, 3]
    # w2s[d, 8t'+o] = prev-update weight from h(t') feature d to output o;
    # t-invariant blocks, accumulated incrementally on-chip.
    w2 = np.zeros((8, DIM), np.float32)
    for tp in range(T):
        w2[0:7, tp * 8:tp * 8 + 7] = his_weight[:, 7 * tp:7 * tp + 7].T
        w2[7, tp * 8 + 7] = cur_weight[0, tp]
    # interleaved feature (8t+d) -> reference feature (7t+d | 84+t)
    f_ref = np.array([7 * t + d if d < 7 else 84 + t
                      for t in range(T) for d in range(8)])
    wf96 = final_weight[:, f_ref].T  # [96 (8t+d), 96 (out)]
    # wf3[d, t*96+o] = wf96[8t+d, o]
    wf = np.ascontiguousarray(
        wf96.reshape(T, 8, DIM).transpose(1, 0, 2).reshape(8, T * DIM))
    # merged lhsT blocks:
    # wrp [104, 96]: rows 0:96 = wraw block-diag, rows 96:104 = wprev
    wrp = np.concatenate([wraw, wprev], axis=0)
    # wcomb [8, T*104]: per t, cols 0:96 = wf block(t), cols 96:104 = w2s(t)
    wcomb = np.zeros((8, T, 104), np.float32)
    for t in range(T):
        wcomb[:, t, 0:DIM] = wf[:, t * DIM:(t + 1) * DIM]
        wcomb[:, t, DIM:104] = w2[:, t * 8:(t + 1) * 8]
    wcomb = np.ascontiguousarray(wcomb.reshape(8, T * 104))

    in_maps = []
    for c in range(NCORES):
        c0 = c * NPC
        ac = np.zeros((NK, NP), BF16_NP)
        ac[:N, :NPC] = adjT[:, c0:c0 + NPC]
        # a[h, p, k, n] = ac[k*KT+p, h*NH+n]
        ah = np.ascontiguousarray(
            ac.reshape(NKT, KT, 2, NH).transpose(2, 1, 0, 3))
        dtc = np.zeros((DIM, NP), np.float32)
        dtc[:, :NPC] = data[:, c0:c0 + NPC, :].transpose(0, 2, 1).reshape(
            DIM, NPC)
        ptc = np.zeros((8, T, NP), np.float32)
        ptc[:, :, :NPC] = pos[:, c0:c0 + NPC, :].transpose(2, 0, 1)
        in_maps.append({
            "a": ah, "xe": xe_h, "dt": dtc.astype(BF16_NP), "pt": ptc,
            "wrp": wrp.astype(BF16_NP), "wagg": wagg.astype(BF16_NP),
            "wcomb": wcomb.astype(BF16_NP),
        })
    return in_maps


def assemble(results):
    out = np.empty((N, DIM), np.float32)
    for c in range(NCORES):
        out[c * NPC:(c + 1) * NPC, :] = results[c]["out"][:, :NPC].T
    return out


_NC_CACHE = None


def get_nc():
    global _NC_CACHE
    if _NC_CACHE is None:
        _NC_CACHE = build_nc()
    return _NC_CACHE


def run_spmd(in_maps, **kwargs):
    nc = get_nc()
    return bass_utils.run_bass_kernel_spmd(
        nc, in_maps, list(range(NCORES)), **kwargs)


def kernel(**inputs):
    in_maps = prep_in_maps(**inputs)
    res = run_spmd(in_maps)
    return assemble(res.results)


# revision 46
# speedup vs baseline: 1.3994x; 1.0081x over previous
"""Trainium2 Bass kernel for nn_CombinedGNN (gnn_message_passing).

Strategy (8 NeuronCores, node/row parallel, zero collectives):
  - masks[1] in the reference is identically zero (elementwise pow of a 0/1
    matrix), so only mask0 = adj/rowdeg matters.
  - All T=12 timesteps' aggregations are mask0 @ data[t] -> batched into ONE
    matmul  adj @ [X | 1]  with X = data rearranged to [N, 96]; the ones
    column yields row degrees, and the 1/deg row scaling is applied after.
  - Each core owns 625 nodes (padded to 640). It gets adj^T's column block
    (so the contraction dim sits on SBUF partitions with contiguous DMA) and
    computes its nodes' full output independently.
  - The sequential t-chain (his_prev/cur_prev recurrences) runs in
    [feature-on-partition, node-on-free] orientation with host-prepacked /
    permuted weight matrices so no on-chip transposes are needed.
  - adj (exactly representable 0/1) and X are cast to bf16 for the big
    matmul; accumulation is fp32 in PSUM. Everything downstream is fp32.
"""

import numpy as np
import ml_dtypes

import concourse.bass as bass
import concourse.mybir as mybir
import concourse.bass_utils as bass_utils
from concourse.tile import TileContext

# problem constants (hardcoded per harness contract)
N, T, DAY, L = 5000, 12, 8, 2
F = DAY - 1
DIM = T * DAY  # 96
NCORES = 8
NPC = N // NCORES        # 625 nodes per core
NP = 640                 # padded nodes per core
NH = NP // 2             # 320, node half processed per psum chunk
KT = 128                 # contraction tile (partitions; K padded to 5120)
NK = 5120                # padded contraction size
NKT = NK // KT           # 40
KG = 20                  # k-tiles per DMA group
NG = NKT // KG           # 2
XW = DIM + 1             # 97: 96 features + ones column

F32 = mybir.dt.float32
BF16 = mybir.dt.bfloat16
BF16_NP = ml_dtypes.bfloat16

_MAXW = 1


def split_multi_waits(nc):
    """Walrus in this container rejects instructions with >~2 sync waits.
    Hoist extra waits onto preceding single-wait NoOps on the same engine."""
    f = nc.m.functions[0]
    for bb in list(f.blocks):
        new, ctr = [], 0
        for inst in bb.instructions:
            si = inst.sync_info
            waits = list(si.on_wait) if (si and si.on_wait) else []
            if len(waits) > _MAXW:
                head, keep = waits[:-_MAXW], waits[-_MAXW:]
                for i in range(0, len(head), _MAXW):
                    nop = mybir.InstNoOp(
                        name=f"{inst.name}-wsplit{ctr}", engine=inst.engine,
                        ins=[], outs=[],
                        sync_info=mybir.SyncInfo(on_wait=head[i:i + _MAXW],
                                                 on_update=[]),
                    )
                    ctr += 1
                    new.append(nop)
                inst.sync_info = mybir.SyncInfo(
                    on_wait=keep,
                    on_update=list(si.on_update) if si.on_update else [])
            new.append(inst)
        bb.instructions = new


def build_nc():
    nc = bass.Bass()
    a_d = nc.dram_tensor("a", [2, KT, NKT, NH], BF16, kind="ExternalInput")
    xe_d = nc.dram_tensor("xe", [KT, NKT, XW], BF16, kind="ExternalInput")
    dt_d = nc.dram_tensor("dt", [8, T, NP], BF16, kind="ExternalInput")
    pt_d = nc.dram_tensor("pt", [8, T, NP], F32, kind="ExternalInput")
    # w1: [24, 96] — per t, rows 0:8 prev-block, 8:16 raw, 16:24 agg
    w1_d = nc.dram_tensor("w1", [24, DIM], BF16, kind="ExternalInput")
    # wcomb: [8, T, 104] — cols 0:96 wf block(t), cols 96:104 w2s block(t)
    wcomb_d = nc.dram_tensor("wcomb", [8, T * 104], BF16,
                             kind="ExternalInput")
    out_d = nc.dram_tensor("out", [DIM, NP], F32, kind="ExternalOutput")

    with TileContext(nc) as tc:
        with (
            tc.tile_pool(name="const", bufs=1) as cpool,
            tc.tile_pool(name="adma", bufs=3) as apool,
            tc.tile_pool(name="work", bufs=2) as wpool,
            tc.tile_pool(name="pagg", bufs=2, space="PSUM") as pagg,
            tc.tile_pool(name="pbc", bufs=1, space="PSUM") as pbc,
            tc.tile_pool(name="pp1", bufs=3, space="PSUM") as pp1,
            tc.tile_pool(name="pcm", bufs=2, space="PSUM") as pcm,
        ):
            # SP HWDGE ring order: xe chunk0, a(A,0), xe rest, a(A,1),
            # a(B,*) — phase-1A matmuls can start ~4us in; remaining
            # consts ride the ACT HWDGE ring.
            xe_t = cpool.tile([KT, NKT, XW], BF16)
            a_tiles = {}
            def a_dma(h, g):
                a_t = apool.tile([KT, KG, NH], BF16, tag="a",
                                 name=f"a{h}{g}")
                nc.sync.dma_start(
                    out=a_t, in_=a_d[h, :, g * KG:(g + 1) * KG, :])
                a_tiles[(h, g)] = a_t
            nc.sync.dma_start(out=xe_t[:, 0:KG, :], in_=xe_d[:, 0:KG, :])
            nc.sync.dma_start(out=xe_t[:, KG:NKT, :], in_=xe_d[:, KG:NKT, :])
            a_dma(0, 0)
            a_dma(0, 1)
            a_dma(1, 0)
            a_dma(1, 1)
            # dag: per-t matmul rhs [24, T, NP] — rows 0:8 prev state
            # (written by the chain relu), 8:16 dataT, 16:24 scaled agg
            dag_t = cpool.tile([24, T, NP], BF16)
            nc.scalar.dma_start(out=dag_t[8:16, :, :], in_=dt_d[:, :, :])
            nc.vector.memset(dag_t[0:8, 0, :], 0.0)
            pt_t = cpool.tile([8, T, NP], F32)
            nc.scalar.dma_start(out=pt_t, in_=pt_d[:, :, :])
            w1_t = cpool.tile([24, DIM], BF16)
            nc.scalar.dma_start(out=w1_t, in_=w1_d[:, :])
            wcomb_t = cpool.tile([8, T * 104], BF16)
            nc.scalar.dma_start(out=wcomb_t, in_=wcomb_d[:, :])

            ones_t = cpool.tile([1, DIM], F32)
            nc.vector.memset(ones_t, 1.0)
            h2_t = cpool.tile([8, T, NP], BF16)
            aggs_t = cpool.tile([DIM, NP], BF16)
            outt_t = cpool.tile([DIM, NP], F32)

            # phase 1: aggT[97, NH] = [X|1]^T @ adjT_shard  per node half
            def phase1(h):
                for g in range(NG):
                    a_t = a_tiles[(h, g)]
                    for j in range(KG):
                        k = g * KG + j
                        nc.tensor.matmul(aggp_t[h], xe_t[:, k, :],
                                         a_t[:, j, :],
                                         start=(k == 0), stop=(k == NKT - 1))

            # row-normalization: broadcast max(deg,1) to 96 partitions via
            # a tiny matmul, then divide (avoids the slow DVE reciprocal).
            def transition(h):
                cs = slice(h * NH, (h + 1) * NH)
                aggp = aggp_t[h]
                degm = wpool.tile([1, NH], F32, tag="degm", name=f"degm{h}")
                nc.vector.tensor_scalar_max(degm, aggp[DIM:DIM + 1, :], 1.0)
                rb_p = pbc.tile([DIM, NH], F32, tag="rbp", name=f"rbp{h}")
                nc.tensor.matmul(rb_p, ones_t, degm, start=True, stop=True)
                rb_s = wpool.tile([DIM, NH], F32, tag="rbs", name=f"rbs{h}")
                nc.vector.reciprocal(rb_s, rb_p)
                nc.vector.tensor_mul(aggs_t[:, cs], aggp[0:DIM, :], rb_s)
                # scatter scaled agg rows (8t+d) -> dag rows 16+d, slab t
                for t in range(T):
                    nc.sync.dma_start(
                        out=dag_t[16:24, t, cs],
                        in_=aggs_t[t * 8:(t + 1) * 8, cs])

            # phase 2: sequential t-chain for one node half.
            # Per t: p1 = wagg[t]^T @ aggs + w_rp[t]^T @ [dt; prev] (chain);
            # pcomb += wcomb[t]^T @ h(t)  (rows 0:96 final acc, 96:104 prev
            # acc, read mid-group by the relu).
            def chain_step(h, t, pcomb):
                cs = slice(h * NH, (h + 1) * NH)
                r8 = slice(t * 8, t * 8 + 8)
                p1 = pp1.tile([8, NH], F32, tag="p1", name=f"p1_{h}_{t}")
                nc.tensor.matmul(p1, w1_t[:, r8], dag_t[:, t, cs],
                                 start=True, stop=True)
                # h(t) = relu(p1) + pos(t)   (fused on DVE)
                nc.vector.scalar_tensor_tensor(
                    h2_t[:, t, cs], p1, 0.0, pt_t[:, t, cs],
                    op0=mybir.AluOpType.max, op1=mybir.AluOpType.add)
                nc.tensor.matmul(pcomb,
                                 wcomb_t[:, t * 104:(t + 1) * 104],
                                 h2_t[:, t, cs],
                                 start=(t == 0), stop=(t == T - 1),
                                 skip_group_check=True)
                # prev = relu(p2 rows) -> next slab  (ScalarE, off the DVE)
                if t < T - 1:
                    nc.scalar.activation(
                        dag_t[0:8, t + 1, cs], pcomb[DIM:104, :],
                        mybir.ActivationFunctionType.Relu)

            def final(h, pcomb):
                cs = slice(h * NH, (h + 1) * NH)
                nc.scalar.activation(outt_t[:, cs], pcomb[0:DIM, :],
                                     mybir.ActivationFunctionType.Relu)
                nc.sync.dma_start(out=out_d[:, cs], in_=outt_t[:, cs])

            # emission order = scheduler priority: chain A is emitted
            # before phase-1B matmuls so the PE stream interleaves B's
            # matmuls into chain-A's dependency gaps.
            aggp_t = [pagg.tile([XW, NH], F32, tag="aggp", name=f"aggp{h}")
                      for h in range(2)]
            pcombs = [pcm.tile([104, NH], F32, tag="pcm", name=f"pcomb{h}")
                      for h in range(2)]
            phase1(0)
            transition(0)
            phase1(1)
            transition(1)
            for t in range(T):
                chain_step(0, t, pcombs[0])
                chain_step(1, t, pcombs[1])
            final(0, pcombs[0])
            final(1, pcombs[1])

    split_multi_waits(nc)
    return nc


def prep_in_maps(adj, data, pos, his_W, cur_W, his_weight, cur_weight,
                 final_weight):
    adj = np.asarray(adj, dtype=np.float32)
    data = np.asarray(data, dtype=np.float32)
    pos = np.asarray(pos, dtype=np.float32)
    his_W = np.asarray(his_W, dtype=np.float32)
    cur_W = np.asarray(cur_W, dtype=np.float32)
    his_weight = np.asarray(his_weight, dtype=np.float32)
    cur_weight = np.asarray(cur_weight, dtype=np.float32)
    final_weight = np.asarray(final_weight, dtype=np.float32)

    # X = data rearranged [N, 96] (col = t*8+d), plus ones column;
    # contraction dim zero-padded to NK=5120 for full-128-partition tiles
    X = np.ascontiguousarray(data.transpose(1, 0, 2).reshape(N, DIM))
    Xe = np.zeros((NK, XW), np.float32)
    Xe[:N, :DIM] = X
    Xe[:N, DIM] = 1.0
    # pre-tiled for DMA: xe[p, k, c] = Xe[k*KT+p, c]
    xe_h = np.ascontiguousarray(
        Xe.reshape(NKT, KT, XW).transpose(1, 0, 2)).astype(BF16_NP)

    adjT = np.ascontiguousarray(adj.T).astype(BF16_NP)

    # weight packing (zero-padded block maps, see build_nc layout)
    # w1 [24, 96]: per-t lhsT for the merged p1 matmul over dag rows
    # [prev(8); raw(8); agg(8)]
    w1 = np.zeros((24, DIM), np.float32)
    for t in range(T):
        w1[0:7, t * 8:t * 8 + 7] = his_W[t][:, 21:28].T
        w1[7, t * 8 + 7] = cur_W[t][0, 3]
        w1[8:15, t * 8:t * 8 + 7] = his_W[t][:, 0:7].T
        w1[15, t * 8 + 7] = cur_W[t][0, 0]
        w1[16:23, t * 8:t * 8 + 7] = his_W[t][:, 7:14].T
        w1[23, t * 8 + 7] = cur_W[t][0, 1]
    # w2s[d, 8t'+o] = prev-update weight from h(t') feature d to output o;
    # t-invariant blocks, accumulated incrementally on-chip.
    w2 = np.zeros((8, DIM), np.float32)
    for tp in range(T):
        w2[0:7, tp * 8:tp * 8 + 7] = his_weight[:, 7 * tp:7 * tp + 7].T
        w2[7, tp * 8 + 7] = cur_weight[0, tp]
    # interleaved feature (8t+d) -> reference feature (7t+d | 84+t)
    f_ref = np.array([7 * t + d if d < 7 else 84 + t
                      for t in range(T) for d in range(8)])
    wf96 = final_weight[:, f_ref].T  # [96 (8t+d), 96 (out)]
    # wf3[d, t*96+o] = wf96[8t+d, o]
    wf = np.ascontiguousarray(
        wf96.reshape(T, 8, DIM).transpose(1, 0, 2).reshape(8, T * DIM))
    # wcomb [8, T*104]: per t, cols 0:96 = wf block(t), cols 96:104 = w2s(t)
    wcomb = np.zeros((8, T, 104), np.float32)
    for t in range(T):
        wcomb[:, t, 0:DIM] = wf[:, t * DIM:(t + 1) * DIM]
        wcomb[:, t, DIM:104] = w2[:, t * 8:(t + 1) * 8]
    wcomb = np.ascontiguousarray(wcomb.reshape(8, T * 104))

    in_maps = []
    for c in range(NCORES):
        c0 = c * NPC
        ac = np.zeros((NK, NP), BF16_NP)
        ac[:N, :NPC] = adjT[:, c0:c0 + NPC]
        # a[h, p, k, n] = ac[k*KT+p, h*NH+n]
        ah = np.ascontiguousarray(
            ac.reshape(NKT, KT, 2, NH).transpose(2, 1, 0, 3))
        dtc = np.zeros((8, T, NP), np.float32)
        dtc[:, :, :NPC] = data[:, c0:c0 + NPC, :].transpose(2, 0, 1)
        ptc = np.zeros((8, T, NP), np.float32)
        ptc[:, :, :NPC] = pos[:, c0:c0 + NPC, :].transpose(2, 0, 1)
        in_maps.append({
            "a": ah, "xe": xe_h, "dt": dtc.astype(BF16_NP), "pt": ptc,
            "w1": w1.astype(BF16_NP), "wcomb": wcomb.astype(BF16_NP),
        })
    return in_maps


def assemble(results):
    out = np.empty((N, DIM), np.float32)
    for c in range(NCORES):
        out[c * NPC:(c + 1) * NPC, :] = results[c]["out"][:, :NPC].T
    return out


_NC_CACHE = None


def get_nc():
    global _NC_CACHE
    if _NC_CACHE is None:
        _NC_CACHE = build_nc()
    return _NC_CACHE


def run_spmd(in_maps, **kwargs):
    nc = get_nc()
    return bass_utils.run_bass_kernel_spmd(
        nc, in_maps, list(range(NCORES)), **kwargs)


def kernel(**inputs):
    in_maps = prep_in_maps(**inputs)
    res = run_spmd(in_maps)
    return assemble(res.results)
